# revision 23
# baseline (speedup 1.0000x reference)
"""CAM (channel attention) module kernel for Trainium2, data-parallel over batch.

Computes, per sample:
    v = x.reshape(C, N)                  # N = H*W
    energy = v @ v.T                     # [C, C]
    att = softmax(rowmax(energy) - energy, axis=-1)
    out = gamma * (att @ v) + x

Distribution: batch B=32 split over 8 NeuronCores (4 samples/core), gamma
replicated.  Per core everything is computed on-chip:
  - v loaded once to SBUF (doubles as x for the residual add)
  - v^T built with PE transpose-mode matmuls (needed for the energy matmul,
    whose contraction is over the spatial axis)
  - energy accumulated in PSUM with float32r (FP22) matmuls: full bf16-rate
    (1 cycle/row) with ~2^-12 operand rounding
  - energy is SYMMETRIC: row-block ib only computes columns >= ib*128 (row 3
    widened to 256 cols since fp32r matmuls under 256 free-dim run at 1/4
    rate); the 5 remaining lower sub-blocks are mirrored with [128,128] PE
    transposes of upper sub-blocks (copied PSUM->SBUF as fp32r first, as the
    BIR verifier requires fp32r matmul operands to be produced as fp32r).
    Cuts energy matmul cycles by 31%.  The mirrors join each bank's
    accumulation group (start=False, carrying the final stop) since PSUM
    allows only one pending group per bank.
  - each energy block gets its OWN single-bank PSUM tile: a multi-bank tile
    serializes every block's start behind the previous block's exp (the
    tile-granular start-group guard), putting ACT on the PE critical path
  - sample 0 has no previous sample to hide its v^T transposes under, and is
    DMA-bound at startup anyway: it uses a k-outer full (non-triangular)
    energy loop so each spatial chunk feeds all 4 row blocks as soon as it
    lands, matching the DMA delivery rate
  - softmax via the identity softmax(rowmax - e) = exp(rowmin - e)/sum(...):
    row-min on DVE, exp (+ fused row-sum) on ACT
  - unnormalized attention transposed with 16 PE transposes, so the second
    matmul can contract over the attention column axis; the row
    normalization (1/Z) and gamma are folded into a single per-partition
    scalar applied in the epilogue
  - epilogue fuses (psum * (gamma/Z)) + x in one DVE pass, writing bf16:
    halves the HBM write traffic for ~2^-9 relative rounding (the kernel is
    near the DMA roofline; output is upcast to fp32 on the host)
"""

import sys

sys.path.insert(0, "/opt/trn_rl_repo")

from contextlib import ExitStack

import numpy as np

import concourse.bacc as bacc
import concourse.bass as bass
import concourse.mybir as mybir
import concourse.tile as tile
from concourse import masks
from concourse.bass_utils import run_bass_kernel_spmd

B, C, H, W = 32, 512, 48, 48
N = H * W  # 2304
NCORES = 8
SPC = B // NCORES  # samples per core
P = 128
CB = C // P  # 4 channel blocks
KB = N // P  # 18 spatial chunks of 128
NCH = [512, 512, 512, 512, 256]  # free-dim chunking of N for the 2nd matmul

FP32 = mybir.dt.float32
FP32R = mybir.dt.float32r
BF16 = mybir.dt.bfloat16
AX = mybir.AxisListType.X
OP = mybir.AluOpType
AF = mybir.ActivationFunctionType


def _emit(tc, ctx, x, gamma, out, reps=1):
    nc = tc.nc

    const_pool = ctx.enter_context(tc.tile_pool(name="const", bufs=1))
    ident_f32 = const_pool.tile([P, P], FP32)
    masks.make_identity(nc, ident_f32[:])
    ident = const_pool.tile([P, P], FP32R)
    nc.scalar.copy(ident[:], ident_f32[:])
    gamma_sb = const_pool.tile([P, 1], FP32)
    nc.sync.dma_start(gamma_sb[:], bass.AP(gamma.tensor, 0, [[0, P], [1, 1]]))

    v_pool = ctx.enter_context(tc.tile_pool(name="v", bufs=3))
    vt_pool = ctx.enter_context(tc.tile_pool(name="vt", bufs=1))
    p_pool = ctx.enter_context(tc.tile_pool(name="p", bufs=2))
    pt_pool = ctx.enter_context(tc.tile_pool(name="pt", bufs=2))
    m_pool = ctx.enter_context(tc.tile_pool(name="m", bufs=1))
    o_pool = ctx.enter_context(tc.tile_pool(name="o", bufs=3))
    vec_pool = ctx.enter_context(tc.tile_pool(name="vec", bufs=4))
    s_pool = ctx.enter_context(tc.tile_pool(name="s", bufs=2))
    # PSUM budget is exactly 8 banks: energy/attn-T share one 4-bank slot
    # (their lifetimes are disjoint), 2 rotating transpose banks, 2 output
    # banks.
    ps_e = ctx.enter_context(tc.tile_pool(name="ps_e", bufs=1, space="PSUM"))
    ps_t = ctx.enter_context(tc.tile_pool(name="ps_t", bufs=2, space="PSUM"))
    ps_o = ctx.enter_context(tc.tile_pool(name="ps_o", bufs=2, space="PSUM"))

    nsamp = reps * SPC
    v_t = {}
    vt_t = {}

    def load_v(i):
        # column-range-major order so sample-0's just-in-time transposes can
        # start as soon as the first range lands on all 4 channel blocks
        s = i % SPC
        v = v_pool.tile([P, CB * N], FP32R, tag="v", name=f"v{i}")
        for a, b in ((0, 256), (256, 768), (768, 1536), (1536, N)):
            for cb in range(CB):
                nc.sync.dma_start(
                    v[:, cb * N + a : cb * N + b],
                    x[s, cb * P : (cb + 1) * P, a:b].bitcast(FP32R),
                )
        v_t[i] = v

    def a_chunk(i, k, copy_eng=None):
        # transpose one 128-wide spatial chunk of v into vt
        if k == 0:
            vt_t[i] = vt_pool.tile([P, KB * C], FP32R, tag="vt", name=f"vt{i}")
        v, vt = v_t[i], vt_t[i]
        tps = ps_t.tile([P, 512], FP32R, tag="tps")
        for cb in range(CB):
            nc.tensor.matmul(
                tps[:, cb * P : (cb + 1) * P],
                v[:, cb * N + k * P : cb * N + (k + 1) * P],
                ident[:],
                is_transpose=True,
                start=(cb == 0),
                stop=(cb == CB - 1),
            )
        if copy_eng == "dve":
            nc.vector.tensor_copy(vt[:, k * C : (k + 1) * C], tps[:])
        elif copy_eng == "act":
            nc.scalar.copy(vt[:, k * C : (k + 1) * C], tps[:])
        else:
            nc.any.tensor_copy(vt[:, k * C : (k + 1) * C], tps[:])

    def emit(i):
        s = i % SPC
        v, vt = v_t[i], vt_t.get(i)
        # depth-2 input prefetch: each sample's load gets ~two sample periods
        # to stream, so the next sample's transposes never wait on DMA.
        # Sample 0 is DMA-bound already — its successors defer to 'out' phase.
        if i == 0:
            if nsamp > 1:
                load_v(1)
        elif i + 2 < nsamp:
            load_v(i + 2)
        # one tile per PSUM bank so a block's start-group only guards its own
        # bank (a single 4-bank tile serializes each block's matmuls behind
        # the previous block's exp)
        energy = [
            ps_e.tile([P, 512], FP32, tag=f"eb{b}", name=f"energy{i}_{b}")
            for b in range(CB)
        ]
        p_sb = p_pool.tile([P, CB * 512], FP32R, tag="p")
        s_all = s_pool.tile([P, CB], FP32, tag="s")

        def softmax_block(ib):
            # softmax(rowmax - e) == exp(rowmin - e) / rowsum
            e_ib = energy[ib][:]
            mn = vec_pool.tile([P, 1], FP32, tag="mn")
            nc.vector.tensor_reduce(mn[:], e_ib, axis=AX, op=OP.min)
            z = vec_pool.tile([P, 1], FP32, tag="z")
            nc.scalar.activation(
                p_sb[:, ib * 512 : (ib + 1) * 512],
                e_ib,
                AF.Exp,
                bias=mn[:],
                scale=-1.0,
                accum_out=z[:],
            )
            r = vec_pool.tile([P, 1], FP32, tag="r")
            nc.vector.reciprocal(r[:], z[:])
            nc.vector.tensor_tensor(
                s_all[:, ib : ib + 1], r[:], gamma_sb[:], op=OP.mult
            )

        if i == 0:
            # ---- sample 0: full energy, k-outer (DMA-rate-matched) ----
            for k in range(KB):
                if k == 0:
                    a_chunk(0, 0)
                if k + 1 < KB:
                    a_chunk(0, k + 1)
                vt = vt_t[0]
                for ib in range(CB):
                    nc.tensor.matmul(
                        energy[ib][:],
                        vt[:, k * C + ib * P : k * C + (ib + 1) * P],
                        vt[:, k * C : (k + 1) * C],
                        start=(k == 0),
                        stop=(k == KB - 1),
                    )
            for ib in range(CB):
                softmax_block(ib)
        else:
            # ---- triangular energy: block row ib computes cols >= lo[ib];
            # the sub-128-wide remainder is mirrored by transposing upper
            # sub-blocks (fp32r matmuls under 256 free run at 1/4 rate, so
            # row 3 recomputes cols [256,384) rather than mirroring them).
            # Mirrors join each bank's accumulation group (one pending group
            # per PSUM bank): start=False, stop carried by the last mirror.
            lo = [0, P, 2 * P, 2 * P]
            mirrors = {1: [0], 2: [0, 1], 3: [0, 1]}
            m_t = {}
            for ib in range(CB):
                for k in range(KB):
                    nc.tensor.matmul(
                        energy[ib][:, lo[ib] : 512],
                        vt[:, k * C + ib * P : k * C + (ib + 1) * P],
                        vt[:, k * C + lo[ib] : (k + 1) * C],
                        start=(k == 0),
                        stop=(k == KB - 1 and not mirrors.get(ib)),
                    )
                for j, src in enumerate(mirrors.get(ib, ())):
                    nc.tensor.matmul(
                        energy[ib][:, src * P : (src + 1) * P].bitcast(FP32R),
                        m_t[src][:, (ib - src - 1) * P : (ib - src) * P],
                        ident[:],
                        is_transpose=True,
                        start=False,
                        stop=(j == len(mirrors[ib]) - 1),
                    )
                if ib < 2:
                    # export sub-blocks for later rows (one copy per source);
                    # fp32r tile so the copy itself rounds for the transpose
                    m = m_pool.tile([P, 512 - (ib + 1) * P], FP32R, tag=f"m{ib}")
                    nc.vector.tensor_copy(
                        m[:], energy[ib][:, (ib + 1) * P : 512]
                    )
                    m_t[ib] = m
                softmax_block(ib)

        # a few of the next sample's transposes fill the exp tail (copies on
        # DVE: ACT is busy with the exps here)
        if i + 1 < nsamp:
            a_queue = list(range(KB))
            for _ in range(3):
                a_chunk(i + 1, a_queue.pop(0), copy_eng="dve")
        else:
            a_queue = []

        # ---- transpose unnormalized attention: PT[d, c] = P[c, d] ----
        # reuses the energy banks (tags "eb*"): 16 blocks, one group per bank
        pt_ps = [
            ps_e.tile([P, 512], FP32R, tag=f"eb{b}", name=f"pt_ps{i}_{b}")
            for b in range(CB)
        ]
        for cb in range(CB):
            for db in range(CB):
                nc.tensor.matmul(
                    pt_ps[db][:, cb * P : (cb + 1) * P],
                    p_sb[:, cb * 512 + db * P : cb * 512 + (db + 1) * P],
                    ident[:],
                    is_transpose=True,
                    start=(cb == 0),
                    stop=(cb == CB - 1),
                )
        pt_sb = pt_pool.tile([P, CB * 512], FP32R, tag="pt")
        for db in range(CB):
            nc.vector.tensor_copy(
                pt_sb[:, db * 512 : (db + 1) * 512],
                pt_ps[db][:],
            )

        # ---- out = (PT^T @ v) * (gamma/Z) + x, next-sample transposes mixed in
        if i == 0 and nsamp > 2:
            load_v(2)
        for cb in range(CB):
            n_off = 0
            for nch in NCH:
                if a_queue:
                    # copies on ACT: DVE is busy with the epilogue here
                    a_chunk(i + 1, a_queue.pop(0), copy_eng="act")
                po = ps_o.tile([P, 512], FP32, tag="po")
                for db in range(CB):
                    nc.tensor.matmul(
                        po[:, :nch],
                        pt_sb[:, db * 512 + cb * P : db * 512 + (cb + 1) * P],
                        v[:, db * N + n_off : db * N + n_off + nch],
                        start=(db == 0),
                        stop=(db == CB - 1),
                    )
                # bf16 output: halves HBM write traffic and doubles the DVE
                # epilogue rate; ~2^-9 relative rounding is well inside the
                # error budget
                ot = o_pool.tile([P, 512], BF16, tag="ot")
                nc.vector.scalar_tensor_tensor(
                    ot[:, :nch],
                    po[:, :nch],
                    s_all[:, cb : cb + 1],
                    v[:, cb * N + n_off : cb * N + n_off + nch].bitcast(FP32),
                    op0=OP.mult,
                    op1=OP.add,
                )
                nc.sync.dma_start(
                    out[s, cb * P : (cb + 1) * P, n_off : n_off + nch], ot[:, :nch]
                )
                n_off += nch
        del v_t[i], vt_t[i]

    load_v(0)
    for i in range(nsamp):
        emit(i)


_nc_cache = {}


def _build(reps=1):
    if reps in _nc_cache:
        return _nc_cache[reps]
    nc = bacc.Bacc("TRN2", target_bir_lowering=False, debug=False)
    x_d = nc.dram_tensor("x", [SPC, C, N], FP32, kind="ExternalInput")
    g_d = nc.dram_tensor("gamma", [1], FP32, kind="ExternalInput")
    o_d = nc.dram_tensor("out", [SPC, C, N], BF16, kind="ExternalOutput")
    with tile.TileContext(nc) as tc, ExitStack() as ctx:
        _emit(tc, ctx, x_d.ap(), g_d.ap(), o_d.ap(), reps=reps)
    nc.compile()
    _nc_cache[reps] = nc
    return nc


def _bench_fn(reps, x, gamma):
    """Build a jitted 8-core executor for the reps-times-repeated kernel with
    device-resident inputs.  Used by test.py for differential timing."""
    import jax
    from jax.experimental.shard_map import shard_map
    from jax.sharding import Mesh, NamedSharding, PartitionSpec

    from concourse import bass2jax

    bass2jax.install_neuronx_cc_hook()
    nc = _build(reps=reps)
    pid = nc.partition_id_tensor.name if nc.partition_id_tensor else None
    in_names, out_names, out_avals, zero_outs = [], [], [], []
    for alloc in nc.m.functions[0].allocations:
        if not isinstance(alloc, mybir.MemoryLocationSet):
            continue
        name = alloc.memorylocations[0].name
        if alloc.kind == "ExternalInput":
            if name != pid:
                in_names.append(name)
        elif alloc.kind == "ExternalOutput":
            out_names.append(name)
            shape = tuple(alloc.tensor_shape)
            dtype = mybir.dt.np(alloc.dtype)
            out_avals.append(jax.core.ShapedArray(shape, dtype))
            zero_outs.append(np.zeros(shape, dtype))
    all_in_names = list(in_names) + list(out_names)
    if pid:
        all_in_names.append(pid)

    def _body(*args):
        operands = list(args)
        if pid:
            operands.append(bass2jax.partition_id_tensor())
        return tuple(
            bass2jax._bass_exec_p.bind(
                *operands,
                out_avals=tuple(out_avals),
                in_names=tuple(all_in_names),
                out_names=tuple(out_names),
                lowering_input_output_aliases=(),
                sim_require_finite=True,
                sim_require_nnan=True,
                nc=nc,
            )
        )

    devices = jax.devices()[:NCORES]
    mesh = Mesh(np.asarray(devices), ("core",))
    specs = (PartitionSpec("core"),) * (len(in_names) + len(out_names))
    fn = jax.jit(
        shard_map(
            _body,
            mesh=mesh,
            in_specs=specs,
            out_specs=(PartitionSpec("core"),) * len(out_names),
            check_rep=False,
        ),
        keep_unused=True,
    )
    sh = NamedSharding(mesh, PartitionSpec("core"))
    ins = {
        "x": np.ascontiguousarray(x, dtype=np.float32).reshape(B, C, N),
        "gamma": np.tile(np.ascontiguousarray(gamma, dtype=np.float32), (NCORES,)),
    }
    args = [jax.device_put(ins[n], sh) for n in in_names]
    args += [
        jax.device_put(np.zeros((NCORES * z.shape[0], *z.shape[1:]), z.dtype), sh)
        for z in zero_outs
    ]
    return fn, args


def kernel(x: np.ndarray, gamma: np.ndarray, **run_kwargs) -> np.ndarray:
    assert x.shape == (B, C, H, W), x.shape
    nc = _build()
    xr = np.ascontiguousarray(x, dtype=np.float32).reshape(B, C, N)
    g = np.ascontiguousarray(gamma, dtype=np.float32)
    in_maps = [
        {"x": xr[g_idx * SPC : (g_idx + 1) * SPC], "gamma": g}
        for g_idx in range(NCORES)
    ]
    res = run_bass_kernel_spmd(nc, in_maps, core_ids=list(range(NCORES)), **run_kwargs)
    outs = [res.results[g_idx]["out"] for g_idx in range(NCORES)]
    full = np.concatenate(outs, axis=0).reshape(B, C, H, W).astype(np.float32)
    if run_kwargs:
        kernel.last_results = res
    return full
